# revision 6
# baseline (speedup 1.0000x reference)
"""Trainium2 Bass kernel for nn_AttentionNet (B=8, d=256, N=4096, T=1024).

reference:
    scores = einsum('bdn,bdt->bnt', K, Q) / sqrt(d)
    A      = softmax(scores, axis=1)           # over keys N
    R      = einsum('bdn,bnt->bdt', V, A)
    R_     = concat(R, Q, axis=1)              # (B, 2d, T)
    maxAtt = argmax(A, axis=1)                 # (B, T) int32

Sharding: batch B=8 -> one NeuronCore each, fully data parallel.

Per-core plan (two score orientations; no cross-device comms):
  phase A (t on partitions / n on free dim):
      S1^T = Q^T K as Qh^T.Kh + Qh^T.Kl + Ql^T.Kh  (fp16 split pairs,
      fp32 PSUM accumulate; ~7e-7 score error = fp32-noise level, at full
      PE rate instead of fp32's quarter rate)
      -> exp (ACT, accum row sums) -> E
      argmax along free dim via DVE max/max_index -> maxAtt
      row sums -> 1/s  -> transpose + broadcast to r_bcast (128 x T)
  phase B (n on partitions / t on free dim):
      S2 = Kh^T Qh (fp16, ~5e-4 relative on A - plenty)  -> exp -> E2
      A  = E2 * r_bcast  (DVE; rounds to f32r)   -> DMA out (row-contig)
      R += V^T_tile @ A  (fp32r, PSUM-accumulated over the 32 n-tiles)
  maxAtt needs near-fp32 scores (TF32/bf16 flip argmax for near-ties);
  the softmax/R paths only need ~1e-3 relative accuracy.

The fp16 hi/lo splits are computed on host (input packing): Kh = f16(K),
Kl = f16(K - Kh); same for Q. Total DMA in stays ~10.5MB/core.
"""

import numpy as np

import concourse.mybir as mybir
import concourse.tile as tile
from concourse import bacc
from concourse.bass_utils import run_bass_kernel_spmd
from concourse.masks import make_identity

B, D, N, T = 8, 256, 4096, 1024
NT = T // 128  # 8 t-tiles (partition tiles of phase A)
NN = N // 128  # 32 n-tiles (partition tiles of phase B)
NCH = N // 512  # 8 free-dim chunks per phase-A row
F32 = mybir.dt.float32
F32R = mybir.dt.float32r
F16 = mybir.dt.float16
U32 = mybir.dt.uint32
EXP = mybir.ActivationFunctionType.Exp
SCALE = 1.0 / 16.0  # 1/sqrt(d)


def build(reps=1):
    nc = bacc.Bacc()
    Khd = nc.declare_dram_parameter("Kh", [D, N], F16, False)
    Kld = nc.declare_dram_parameter("Kl", [D, N], F16, False)
    Qhd = nc.declare_dram_parameter("Qh", [D, T], F16, False)
    Qld = nc.declare_dram_parameter("Ql", [D, T], F16, False)
    Vd = nc.declare_dram_parameter("V", [D, N], F32, False)
    Qd = nc.declare_dram_parameter("Q", [D, T], F32, False)
    Rd = nc.declare_dram_parameter("R_", [2 * D, T], F32, True)
    Ad = nc.declare_dram_parameter("A", [N, T], F32, True)
    Md = nc.declare_dram_parameter("MX", [T], U32, True)

    with tile.TileContext(nc) as tc:
        with (
            tc.tile_pool(name="inputs", bufs=1) as inp,
            tc.tile_pool(name="epool", bufs=2) as epool,
            tc.tile_pool(name="vstage", bufs=4) as vstage,
            tc.tile_pool(name="e2pool", bufs=3) as e2pool,
            tc.tile_pool(name="apool", bufs=3) as apool,
            tc.tile_pool(name="small", bufs=2) as small,
            tc.tile_pool(name="psx", bufs=4, space="PSUM") as psx,
            tc.tile_pool(name="psr", bufs=4, space="PSUM") as psr,
        ):
            for _rep in range(reps):
                # ---- persistent inputs / constants ----
                qt = inp.tile([128, 2, T], F32)
                qh = inp.tile([128, 2, T], F16)
                ql = inp.tile([128, 2, T], F16)
                kh = inp.tile([128, 2, N], F16)
                kl = inp.tile([128, 2, N], F16)
                for dst, src in ((qt, Qd), (qh, Qhd), (ql, Qld)):
                    nc.sync.dma_start(
                        out=dst, in_=src[:, :].rearrange("(a p) t -> p a t", p=128)
                    )
                for dst, src in ((kh, Khd), (kl, Kld)):
                    nc.sync.dma_start(
                        out=dst, in_=src[:, :].rearrange("(a p) n -> p a n", p=128)
                    )

                ident = inp.tile([128, 128], F32)
                make_identity(nc, ident)
                ones_col = inp.tile([1, 128], F32)
                nc.vector.memset(ones_col, 1.0)

                vtr = inp.tile([128, NN, 256], F32R)  # V^T, n on partitions
                s_all = inp.tile([128, NT], F32)  # softmax denominators
                r_all = inp.tile([128, NT], F32)
                r8s = inp.tile([8, 128], F32)
                r_row = inp.tile([1, T], F32)
                r_bcast = inp.tile([128, T], F32)
                rsb = inp.tile([128, 2, T], F32)  # R rows staged for output

                # ---- phase A: fp16-split scores, argmax, row sums ----
                for ti in range(NT):
                    e = epool.tile([128, N], F32)
                    ps8 = small.tile([128, NCH], F32)
                    tsl = slice(ti * 128, (ti + 1) * 128)
                    for c in range(NCH):
                        csl = slice(c * 512, (c + 1) * 512)
                        psa = psx.tile([128, 512], F32, tag="x")
                        pairs = ((qh, kh), (qh, kl), (ql, kh))
                        for pi, (lt, rt) in enumerate(pairs):
                            for a in range(2):
                                nc.tensor.matmul(
                                    psa,
                                    lt[:, a, tsl],
                                    rt[:, a, csl],
                                    start=(pi == 0 and a == 0),
                                    stop=(pi == 2 and a == 1),
                                )
                        nc.scalar.activation(
                            e[:, csl],
                            psa,
                            EXP,
                            scale=SCALE,
                            accum_out=ps8[:, c : c + 1],
                        )
                    nc.vector.reduce_sum(
                        s_all[:, ti : ti + 1], ps8, axis=mybir.AxisListType.X
                    )
                    m8 = small.tile([128, 8], F32)
                    i8 = small.tile([128, 8], U32)
                    nc.vector.max(m8, e)
                    nc.vector.max_index(i8, m8, e)
                    nc.sync.dma_start(
                        out=Md[tsl].rearrange("(p a) -> p a", a=1),
                        in_=i8[:, 0:1],
                    )

                # ---- V^T via PE transposes (128x128 blocks) ----
                for a in range(2):
                    for nb in range(NN):
                        vblk = vstage.tile([128, 128], F32)
                        nc.sync.dma_start(
                            out=vblk,
                            in_=Vd[a * 128 : (a + 1) * 128, nb * 128 : (nb + 1) * 128],
                        )
                        pst = psx.tile([128, 128], F32, tag="x")
                        nc.tensor.transpose(pst, vblk, ident)
                        nc.scalar.copy(vtr[:, nb, a * 128 : (a + 1) * 128], pst)

                # ---- 1/s, transposed + broadcast across partitions ----
                nc.vector.reciprocal(r_all, s_all)
                psT = psx.tile([128, 128], F32, tag="x")
                nc.tensor.transpose(psT[0:NT, :], r_all, ident)
                nc.vector.tensor_copy(r8s, psT[0:NT, :])
                # partition-gather: (8 x 128) -> (1 x 1024), t = tile*128 + p
                nc.sync.dma_start(
                    out=r_row[0:1, :].rearrange("p (a c) -> p a c", a=8),
                    in_=r8s[:, :].rearrange("a (p c) -> a p c", p=1),
                )
                for tch in range(2):
                    psb = psx.tile([128, 512], F32, tag="x")
                    nc.tensor.matmul(
                        psb,
                        ones_col,
                        r_row[0:1, tch * 512 : (tch + 1) * 512],
                        start=True,
                        stop=True,
                    )
                    nc.vector.tensor_copy(r_bcast[:, tch * 512 : (tch + 1) * 512], psb)

                # ---- phase B: fp16 scores in (n, t) layout, A out, R accum ----
                psR = [
                    psr.tile([128, 512], F32, name=f"psR{j}", tag=f"psR{j}", bufs=1)
                    for j in range(4)
                ]
                for ni in range(NN):
                    nsl = slice(ni * 128, (ni + 1) * 128)
                    e2 = e2pool.tile([128, T], F32)
                    for tch in range(2):
                        csl = slice(tch * 512, (tch + 1) * 512)
                        psb = psx.tile([128, 512], F32, tag="x")
                        for a in range(2):
                            nc.tensor.matmul(
                                psb,
                                kh[:, a, nsl],
                                qh[:, a, csl],
                                start=(a == 0),
                                stop=(a == 1),
                            )
                        nc.scalar.activation(e2[:, csl], psb, EXP, scale=SCALE)
                    asb = apool.tile([128, T], F32R)
                    nc.vector.tensor_mul(asb, e2, r_bcast)
                    nc.sync.dma_start(out=Ad[nsl, :], in_=asb.bitcast(F32))
                    for dm in range(2):
                        for tch in range(2):
                            nc.tensor.matmul(
                                psR[dm * 2 + tch],
                                vtr[:, ni, dm * 128 : (dm + 1) * 128],
                                asb[:, tch * 512 : (tch + 1) * 512],
                                start=(ni == 0),
                                stop=(ni == NN - 1),
                            )

                # ---- outputs: R_ = concat(R, Q) ----
                for dm in range(2):
                    for tch in range(2):
                        nc.vector.tensor_copy(
                            rsb[:, dm, tch * 512 : (tch + 1) * 512], psR[dm * 2 + tch]
                        )
                    nc.sync.dma_start(
                        out=Rd[dm * 128 : (dm + 1) * 128, :], in_=rsb[:, dm, :]
                    )
                for a in range(2):
                    nc.sync.dma_start(
                        out=Rd[256 + a * 128 : 256 + (a + 1) * 128, :], in_=qt[:, a, :]
                    )

    nc.finalize()
    return nc


_NC = None


def _get_nc():
    global _NC
    if _NC is None:
        _NC = build()
    return _NC


def run(in_maps, trace=False):
    return run_bass_kernel_spmd(_get_nc(), in_maps, list(range(B)), trace=trace)


def make_in_maps(K, V, Q):
    K = np.ascontiguousarray(np.asarray(K), dtype=np.float32)
    V = np.ascontiguousarray(np.asarray(V), dtype=np.float32)
    Q = np.ascontiguousarray(np.asarray(Q), dtype=np.float32)
    Kh = K.astype(np.float16)
    Kl = (K - Kh.astype(np.float32)).astype(np.float16)
    Qh = Q.astype(np.float16)
    Ql = (Q - Qh.astype(np.float32)).astype(np.float16)
    return [
        {
            "Kh": Kh[b],
            "Kl": Kl[b],
            "Qh": Qh[b],
            "Ql": Ql[b],
            "V": V[b],
            "Q": Q[b],
        }
        for b in range(B)
    ]


def assemble(results):
    R_ = np.stack([results[b]["R_"] for b in range(B)])
    A = np.stack([results[b]["A"] for b in range(B)])
    mx = np.stack([results[b]["MX"] for b in range(B)]).astype(np.int32)
    return R_, A, mx


def kernel(K, V, Q):
    res = run(make_in_maps(K, V, Q))
    return assemble(res.results)


# revision 11
# speedup vs baseline: 1.5142x; 1.5142x over previous
"""Trainium2 Bass kernel for nn_AttentionNet (B=8, d=256, N=4096, T=1024).

reference:
    scores = einsum('bdn,bdt->bnt', K, Q) / sqrt(d)
    A      = softmax(scores, axis=1)           # over keys N
    R      = einsum('bdn,bnt->bdt', V, A)
    R_     = concat(R, Q, axis=1)              # (B, 2d, T)
    maxAtt = argmax(A, axis=1)                 # (B, T) int32

Sharding: batch B=8 -> one NeuronCore each, fully data parallel.

Per-core plan (two score orientations; no cross-device comms):
  phase A (t on partitions / n on free dim):
      S1^T = Q^T K as Qh^T.Kh + Qh^T.Kl + Ql^T.Kh  (fp16 split pairs,
      fp32 PSUM accumulate; ~7e-7 score error = fp32-noise level, at full
      PE rate instead of fp32's quarter rate)
      -> exp (ACT, accum row sums) -> E
      argmax along free dim via DVE max/max_index -> maxAtt
      row sums -> 1/s  -> transpose + broadcast to r_bcast (128 x T)
  phase B (n on partitions / t on free dim), two t-halves of 512:
      S2 = Kh^T Qh (fp16, ~5e-4 relative on A - plenty)  -> exp -> E2
      A  = E2 * r_bcast  (DVE; rounds to f32r)   -> DMA out (row-contig)
      R += V^T_tile @ A  (fp32r, PSUM-accumulated over the 32 n-tiles)
  maxAtt needs near-fp32 scores (TF32/bf16 flip argmax for near-ties);
  the softmax/R paths only need ~1e-3 relative accuracy.

Host-side input packing: fp16 hi/lo splits of K and Q, and V^T in fp16
(fp16 mantissa fits TF32 exactly, so the on-device f32r cast is exact).
"""

import numpy as np

import concourse.mybir as mybir
import concourse.tile as tile
from concourse import bacc
from concourse.bass_utils import run_bass_kernel_spmd
from concourse.masks import make_identity

B, D, N, T = 8, 256, 4096, 1024
NT = T // 128  # 8 t-tiles (partition tiles of phase A)
NN = N // 128  # 32 n-tiles (partition tiles of phase B)
NCH = N // 512  # 8 free-dim chunks per phase-A row
F32 = mybir.dt.float32
F32R = mybir.dt.float32r
F16 = mybir.dt.float16
U32 = mybir.dt.uint32
EXP = mybir.ActivationFunctionType.Exp
SCALE = 1.0 / 16.0  # 1/sqrt(d)


def build(reps=1):
    nc = bacc.Bacc()
    Khd = nc.declare_dram_parameter("Kh", [D, N], F16, False)
    Kld = nc.declare_dram_parameter("Kl", [D, N], F16, False)
    Qhd = nc.declare_dram_parameter("Qh", [D, T], F16, False)
    Qld = nc.declare_dram_parameter("Ql", [D, T], F16, False)
    Vtd = nc.declare_dram_parameter("Vt", [N, D], F16, False)
    Qd = nc.declare_dram_parameter("Q", [D, T], F32, False)
    Rd = nc.declare_dram_parameter("R_", [2 * D, T], F32, True)
    Ad = nc.declare_dram_parameter("A", [N, T], F32, True)
    Md = nc.declare_dram_parameter("MX", [T], U32, True)

    with tile.TileContext(nc) as tc:
        with (
            tc.tile_pool(name="inputs", bufs=1) as inp,
            tc.tile_pool(name="epool", bufs=2) as epool,
            tc.tile_pool(name="e2pool", bufs=9) as e2pool,
            tc.tile_pool(name="apool", bufs=6) as apool,
            tc.tile_pool(name="small", bufs=2) as small,
            tc.tile_pool(name="psx", bufs=6, space="PSUM") as psx,
            tc.tile_pool(name="psr", bufs=2, space="PSUM") as psr,
        ):
            for _rep in range(reps):
                # ---- persistent inputs / constants ----
                qh = inp.tile([128, 2, T], F16)
                ql = inp.tile([128, 2, T], F16)
                nc.sync.dma_start(
                    out=qh, in_=Qhd[:, :].rearrange("(a p) t -> p a t", p=128)
                )
                nc.sync.dma_start(
                    out=ql, in_=Qld[:, :].rearrange("(a p) t -> p a t", p=128)
                )
                # K loads chunked so mm1 can start after the first chunk
                kh = inp.tile([128, 2, N], F16)
                kl = inp.tile([128, 2, N], F16)
                for c in range(NCH):
                    csl = slice(c * 512, (c + 1) * 512)
                    for dst, src in ((kh, Khd), (kl, Kld)):
                        nc.sync.dma_start(
                            out=dst[:, :, csl],
                            in_=src[:, csl].rearrange("(a p) n -> p a n", p=128),
                        )
                ident = inp.tile([128, 128], F32)
                make_identity(nc, ident)
                ones_col = inp.tile([1, 128], F32)
                nc.vector.memset(ones_col, 1.0)

                s_all = inp.tile([128, NT], F32)  # softmax denominators
                r_all = inp.tile([128, NT], F32)
                r_row = inp.tile([1, T], F32)
                r_bcast = inp.tile([128, T], F32)
                rsb = inp.tile([128, 2, T], F32)  # R rows staged for output

                def emit_argmax(e, tsl):
                    m8 = small.tile([128, 8], F32, name="m8", tag="m8")
                    i8 = small.tile([128, 8], U32, name="i8", tag="i8")
                    nc.vector.max(m8, e)
                    nc.vector.max_index(i8, m8, e)
                    nc.sync.dma_start(
                        out=Md[tsl].rearrange("(p a) -> p a", a=1),
                        in_=i8[:, 0:1],
                    )

                deferred_argmax = []

                # V^T (host-transposed, f16): load + exact cast to f32r.
                # Emitted after the K chunk loads so K wins the DMA queues.
                vt16 = inp.tile([128, NN, 256], F16)
                nc.sync.dma_start(
                    out=vt16, in_=Vtd[:, :].rearrange("(b p) d -> p b d", p=128)
                )
                vtr = inp.tile([128, NN, 256], F32R)
                nc.scalar.copy(vtr, vt16)
                # Q fp32 only needed for the R_ tail
                qt = inp.tile([128, 2, T], F32)
                nc.sync.dma_start(
                    out=qt, in_=Qd[:, :].rearrange("(a p) t -> p a t", p=128)
                )

                # ---- phase A: fp16-split scores, argmax, row sums ----
                for ti in range(NT):
                    e = epool.tile([128, N], F32)
                    ps8 = small.tile([128, NCH], F32)
                    tsl = slice(ti * 128, (ti + 1) * 128)
                    for c in range(NCH):
                        csl = slice(c * 512, (c + 1) * 512)
                        psa = psx.tile([128, 512], F32, tag="x")
                        pairs = ((qh, kh), (qh, kl), (ql, kh))
                        for pi, (lt, rt) in enumerate(pairs):
                            for a in range(2):
                                nc.tensor.matmul(
                                    psa,
                                    lt[:, a, tsl],
                                    rt[:, a, csl],
                                    start=(pi == 0 and a == 0),
                                    stop=(pi == 2 and a == 1),
                                )
                        nc.scalar.activation(
                            e[:, csl],
                            psa,
                            EXP,
                            scale=SCALE,
                            accum_out=ps8[:, c : c + 1],
                        )
                    nc.vector.reduce_sum(
                        s_all[:, ti : ti + 1], ps8, axis=mybir.AxisListType.X
                    )
                    if ti < NT - 2:
                        emit_argmax(e, tsl)
                    else:
                        deferred_argmax.append((e, tsl))

                # ---- phase B: fp16 scores in (n, t) layout, A out, R accum ----
                # two t-halves so the R accumulators take only 2 PSUM banks
                # Q rows of R_ don't depend on compute: write them early
                for a in range(2):
                    nc.sync.dma_start(
                        out=Rd[256 + a * 128 : 256 + (a + 1) * 128, :], in_=qt[:, a, :]
                    )
                LAG = 3
                PRO = 6  # mm1b/exp2 prologue depth covering the r-dance latency
                for tch in range(2):
                    csl = slice(tch * 512, (tch + 1) * 512)
                    psR = [
                        psr.tile([128, 512], F32, name=f"psR{dm}", tag=f"psR{dm}", bufs=1)
                        for dm in range(2)
                    ]
                    e2s = {}
                    asbs = {}

                    def mm1b_exp(ni):
                        psb = psx.tile([128, 512], F32, tag="x")
                        for a in range(2):
                            nc.tensor.matmul(
                                psb,
                                kh[:, a, ni * 128 : (ni + 1) * 128],
                                qh[:, a, csl],
                                start=(a == 0),
                                stop=(a == 1),
                            )
                        e2 = e2pool.tile([128, 512], F32)
                        nc.scalar.activation(e2, psb, EXP, scale=SCALE)
                        e2s[ni] = e2

                    def mm2(ni):
                        for dm in range(2):
                            nc.tensor.matmul(
                                psR[dm],
                                vtr[:, ni, dm * 128 : (dm + 1) * 128],
                                asbs.pop(ni) if dm == 1 else asbs[ni],
                                start=(ni == 0),
                                stop=(ni == NN - 1),
                            )

                    for ni in range(PRO):
                        mm1b_exp(ni)
                    if tch == 0:
                        # 1/s -> row layout -> broadcast tile. Row j of the
                        # transposed reciprocals is produced on partition 0 by
                        # r_all[:, j].T @ I  (avoids a cross-partition DMA).
                        nc.vector.reciprocal(r_all, s_all)
                        for j in range(NT):
                            psj = psx.tile([1, 128], F32, tag="x")
                            nc.tensor.matmul(
                                psj, r_all[:, j : j + 1], ident, start=True, stop=True
                            )
                            nc.vector.tensor_copy(
                                r_row[0:1, j * 128 : (j + 1) * 128], psj
                            )
                        for h in range(2):
                            psb = psx.tile([128, 512], F32, tag="x")
                            nc.tensor.matmul(
                                psb,
                                ones_col,
                                r_row[0:1, h * 512 : (h + 1) * 512],
                                start=True,
                                stop=True,
                            )
                            nc.vector.tensor_copy(
                                r_bcast[:, h * 512 : (h + 1) * 512], psb
                            )
                        for e, tsl in deferred_argmax:
                            emit_argmax(e, tsl)
                        deferred_argmax.clear()
                    for ni in range(NN):
                        nsl = slice(ni * 128, (ni + 1) * 128)
                        if ni >= PRO:
                            mm1b_exp(ni)
                        asb = apool.tile([128, 512], F32R)
                        nc.vector.tensor_mul(asb, e2s.pop(ni), r_bcast[:, csl])
                        nc.sync.dma_start(out=Ad[nsl, csl], in_=asb.bitcast(F32))
                        asbs[ni] = asb
                        if ni >= LAG:
                            mm2(ni - LAG)
                    for ni in range(NN - LAG, NN):
                        mm2(ni)
                    for dm in range(2):
                        nc.vector.tensor_copy(rsb[:, dm, csl], psR[dm])
                    # R rows out per half
                    for dm in range(2):
                        nc.sync.dma_start(
                            out=Rd[dm * 128 : (dm + 1) * 128, csl], in_=rsb[:, dm, csl]
                        )

    nc.finalize()
    return nc


_NC = None


def _get_nc():
    global _NC
    if _NC is None:
        _NC = build()
    return _NC


def run(in_maps, trace=False):
    return run_bass_kernel_spmd(_get_nc(), in_maps, list(range(B)), trace=trace)


def make_in_maps(K, V, Q):
    K = np.ascontiguousarray(np.asarray(K), dtype=np.float32)
    V = np.ascontiguousarray(np.asarray(V), dtype=np.float32)
    Q = np.ascontiguousarray(np.asarray(Q), dtype=np.float32)
    Kh = K.astype(np.float16)
    Kl = (K - Kh.astype(np.float32)).astype(np.float16)
    Qh = Q.astype(np.float16)
    Ql = (Q - Qh.astype(np.float32)).astype(np.float16)
    Vt = np.ascontiguousarray(V.transpose(0, 2, 1)).astype(np.float16)
    return [
        {
            "Kh": Kh[b],
            "Kl": Kl[b],
            "Qh": Qh[b],
            "Ql": Ql[b],
            "Vt": Vt[b],
            "Q": Q[b],
        }
        for b in range(B)
    ]


def assemble(results):
    R_ = np.stack([results[b]["R_"] for b in range(B)])
    A = np.stack([results[b]["A"] for b in range(B)])
    mx = np.stack([results[b]["MX"] for b in range(B)]).astype(np.int32)
    return R_, A, mx


def kernel(K, V, Q):
    res = run(make_in_maps(K, V, Q))
    return assemble(res.results)
